# revision 13
# baseline (speedup 1.0000x reference)
"""Trainium2 Bass kernel for nn_Attention_3315714753146 (gnn_message_passing).

out = (LA*softmax(mask(QK^T*scale)) + LG*adj_masked + LD*exp(-dist_masked)) @ V @ W_out + b_out

Sharding: 8 shards = (4 batches) x (2 query-row halves of 512 rows). Each core
computes its own 512 output rows from full K/V (computed on-device from x).
No collectives; gather on host.

Device algorithm per core (bf16 matmul operands, fp32 PSUM accumulation,
fp32 exp paths):
  qT = wq^T @ xT_own, kT = wk^T @ xT, v = xT^T @ wv (head-major weights)
  per head: dotsT[j,i] = kT_h chunks as lhsT against qT_h
            pT = exp(0.125*dotsT + biasj)   (biasj = ln(LA) | -1e30 -> folds LA + col mask)
            pv[65, i] = [v_h | 1/LA]^T @ pT (row 64 = softmax denominator)
            innerT_h = pv[0:64] * (maski/denom)  via PE outer-product broadcast
  C0 = (adj * LG*maski) + LD*maski*exp(-dist)  (fp32); transposed on PE, col-masked
  cvT = V^T @ C0T  (+ outer(LA/N * colsum(V), 1-maski) for invalid query rows)
  out = (innerT + cvT)^T @ W_out + b_out  (bias via K=1 ones outer into PSUM)
"""

import sys

for _p in ("/root/.axon_site", "/root/.axon_site/_ro/trn_rl_repo",
           "/root/.axon_site/_ro/pypackages"):
    if _p not in sys.path:
        sys.path.append(_p)

import numpy as np
import ml_dtypes

BF = ml_dtypes.bfloat16
HEADS, DH = 8, 64
B, N, D = 4, 1024, 512
NH = 512          # query rows per core
LA = LD = LG = 0.33
SCALE = DH ** -0.5
NEG = -1e30
NCORES = 8
# exact compensation for the bf16-rounded 1/LA ones-column in vaug
INV_LA_BF = float(np.float32(BF(1.0 / LA)))

_CACHE = {}


def _build_nc():
    import concourse.bass as bass
    import concourse.bacc as bacc
    import concourse.tile as tile
    from concourse import mybir
    from concourse.bass import ts
    from concourse.masks import make_identity

    F32 = mybir.dt.float32
    BF16 = mybir.dt.bfloat16
    AF = mybir.ActivationFunctionType
    OP = mybir.AluOpType

    nc = bacc.Bacc()
    xT = nc.declare_dram_parameter("xT", [D, N], BF16, isOutput=False)
    xTq = nc.declare_dram_parameter("xTq", [D, NH], BF16, isOutput=False)
    wq = nc.declare_dram_parameter("wq", [D, D], BF16, isOutput=False)
    wk = nc.declare_dram_parameter("wk", [D, D], BF16, isOutput=False)
    wv = nc.declare_dram_parameter("wv", [D, D], BF16, isOutput=False)
    wout = nc.declare_dram_parameter("wout", [D, D], BF16, isOutput=False)
    adj = nc.declare_dram_parameter("adj", [NH, N], F32, isOutput=False)
    dist = nc.declare_dram_parameter("dist", [NH, N], F32, isOutput=False)
    # cvec columns: [0:8]=biasj, [8:16]=maskj, [16:20]=lnldmi, [20:24]=lgmi
    cvec = nc.declare_dram_parameter("cvec", [128, 24], F32, isOutput=False)
    # rvec (bf16): [0:512]=1-maski, [512:1024]=b_out, [1024:1152]=ones,
    #              [1152:1664]=u_row (LA/N * colsum(V), host-computed)
    rvec = nc.declare_dram_parameter("rvec", [1, 1664], BF16, isOutput=False)
    mi8 = nc.declare_dram_parameter("mi8", [8, NH], F32, isOutput=False)
    out = nc.declare_dram_parameter("out", [NH, D], F32, isOutput=True)

    with tile.TileContext(nc) as tc:
        with (
            tc.tile_pool(name="const", bufs=1) as constp,
            tc.tile_pool(name="pers", bufs=1) as pers,
            tc.tile_pool(name="big", bufs=8) as bigp,
            tc.tile_pool(name="med", bufs=8) as medp,
            tc.tile_pool(name="wpool", bufs=8) as wpool,
            tc.tile_pool(name="stage", bufs=2) as stagep,
            tc.tile_pool(name="outp", bufs=2) as outp,
            tc.tile_pool(name="rows", bufs=2) as rowsp,
            tc.tile_pool(name="ppool", bufs=17) as ppool,
            tc.tile_pool(name="pgen", bufs=4, space="PSUM") as pgen,
            tc.tile_pool(name="pacc", bufs=2, space="PSUM") as pacc,
            tc.tile_pool(name="psml", bufs=2, space="PSUM") as psml,
        ):
            ident = constp.tile([128, 128], F32, name="ident")
            make_identity(nc, ident[:])
            onesb = constp.tile([128, 1], BF16, name="onesb")
            nc.gpsimd.memset(onesb[:], 1.0)
            cv = constp.tile([128, 24], F32, name="cv")
            nc.sync.dma_start(cv[:], cvec[:])
            rv = constp.tile([1, 1664], BF16, name="rv")
            nc.sync.dma_start(rv[:], rvec[:])
            omi_row = rv[0:1, 0:NH]
            bout_row = rv[0:1, NH:NH + D]
            ones_row = rv[0:1, 2 * NH:2 * NH + 128]
            u_row = rv[0:1, 2 * NH + 128:2 * NH + 128 + D]

            # ---- phase A: projections (one consolidated DMA per tensor,
            # q-path loads first so PE starts early) ----
            def load_w(w_dram, nm):
                t = wpool.tile([128, 4, D], BF16, name=f"{nm}_all", tag="w")
                nc.sync.dma_start(t[:], w_dram[:].rearrange("(c p) d -> p c d", p=128))
                return [t[:, c, :] for c in range(4)]

            wq_sb = load_w(wq, "wq")
            xtq_all = medp.tile([128, 4, NH], BF16, name="xtq_all", tag="medx")
            nc.sync.dma_start(xtq_all[:], xTq[:].rearrange("(c p) d -> p c d", p=128))
            xtq = [xtq_all[:, c, :] for c in range(4)]
            wk_sb = load_w(wk, "wk")
            xt_all = pers.tile([128, 4, N], BF16, name="xt_all")
            nc.sync.dma_start(xt_all[:], xT[:].rearrange("(c p) d -> p c d", p=128))
            xt = [xt_all[:, c, :] for c in range(4)]

            qt = []
            for oc in range(4):
                ps = pgen.tile([128, NH], F32, name="psw", tag="w")
                for cc in range(4):
                    nc.tensor.matmul(ps[:], lhsT=wq_sb[cc][:, ts(oc, 128)],
                                     rhs=xtq[cc][:], start=(cc == 0), stop=(cc == 3))
                t = pers.tile([128, NH], BF16, name=f"qt{oc}")
                nc.vector.tensor_copy(t[:], ps[:])
                qt.append(t)

            kt = [bigp.tile([128, N], BF16, name=f"kt{oc}", tag="big") for oc in range(4)]
            for oc in range(4):
                for nn in range(2):
                    ps = pgen.tile([128, NH], F32, name="psw", tag="w")
                    for cc in range(4):
                        nc.tensor.matmul(ps[:], lhsT=wk_sb[cc][:, ts(oc, 128)],
                                         rhs=xt[cc][:, ts(nn, 512)],
                                         start=(cc == 0), stop=(cc == 3))
                    nc.vector.tensor_copy(kt[oc][:, ts(nn, 512)], ps[:])

            wv_sb = load_w(wv, "wv")
            vaug = [pers.tile([128, HEADS * 65], BF16, name=f"vaug{j}") for j in range(8)]
            v_pl = [pers.tile([128, D], BF16, name=f"vpl{j}") for j in range(8)]
            for ncc in range(8):
                ps = pgen.tile([128, NH], F32, name="psw", tag="w")
                for cc in range(4):
                    nc.tensor.matmul(ps[:], lhsT=xt[cc][:, ts(ncc, 128)],
                                     rhs=wv_sb[cc][:], start=(cc == 0), stop=(cc == 3))
                nc.vector.tensor_copy(v_pl[ncc][:], ps[:])
                v3 = vaug[ncc][:].rearrange("p (h e) -> p h e", e=65)
                nc.vector.tensor_copy(v3[:, :, 0:64],
                                      v_pl[ncc][:].rearrange("p (h d) -> p h d", d=64))
                nc.gpsimd.memset(v3[:, :, 64:65], 1.0 / LA)

            # ---- phase B1: head pairs; even/odd dots interleave so the PE
            # packs them into disjoint 64-row groups (base partitions 0/64) ----
            pv_sb = [medp.tile([65, NH], F32, name=f"pvsb{h}", tag="med") for h in range(HEADS)]
            denoms8 = pers.tile([8, NH], F32, name="denoms8")
            for hp in range(4):
                kc = hp
                pts = {0: [], 1: []}
                for jc in range(8):
                    for par in range(2):
                        ko = par * 64
                        dps = pgen.tile([128, NH], F32, name="psw", tag="w")
                        nc.tensor.matmul(dps[:], lhsT=kt[kc][ko:ko + 64, ts(jc, 128)],
                                         rhs=qt[kc][ko:ko + 64, :], start=True, stop=True)
                        p_t = ppool.tile([128, NH], BF16, name="p_t", tag="p")
                        nc.scalar.activation(p_t[:], dps[:], AF.Exp,
                                             bias=cv[:, jc:jc + 1], scale=SCALE)
                        pts[par].append(p_t)
                for par in range(2):
                    h = 2 * hp + par
                    pvps = pacc.tile([65, NH], F32, name="pvps", tag="a")
                    for jc in range(8):
                        nc.tensor.matmul(pvps[:], lhsT=vaug[jc][:, h * 65:(h + 1) * 65],
                                         rhs=pts[par][jc][:], start=(jc == 0), stop=(jc == 7))
                    nc.vector.tensor_copy(pv_sb[h][:], pvps[:])
                    nc.sync.dma_start(denoms8[h:h + 1, :], pv_sb[h][64:65, :])

            # ---- phase C0 prep (fp32 exp path) ----
            c0 = [bigp.tile([128, N], F32, name=f"c0{ic}", tag="big") for ic in range(4)]
            for ic in range(4):
                adj_t = stagep.tile([128, N], F32, name="adj_t", tag="adj")
                nc.sync.dma_start(adj_t[:], adj[ts(ic, 128), :])
                dist_t = stagep.tile([128, N], F32, name="dist_t", tag="dist")
                nc.sync.dma_start(dist_t[:], dist[ts(ic, 128), :])
                e_t = stagep.tile([128, N], F32, name="e_t", tag="e")
                nc.scalar.activation(e_t[:], dist_t[:], AF.Exp,
                                     bias=cv[:, 16 + ic:17 + ic], scale=-1.0)
                nc.vector.scalar_tensor_tensor(out=c0[ic][:], in0=adj_t[:],
                                               scalar=cv[:, 20 + ic:21 + ic],
                                               in1=e_t[:], op0=OP.mult, op1=OP.add)

            # ---- phase B2: normalize + mask_i via PE outer broadcast ----
            mi8_sb = constp.tile([8, NH], F32, name="mi8_sb")
            nc.sync.dma_start(mi8_sb[:], mi8[:])
            rec8 = pers.tile([8, NH], F32, name="rec8")
            nc.vector.reciprocal(rec8[:], denoms8[:])
            r8 = pers.tile([8, NH], BF16, name="r8")
            nc.vector.tensor_mul(r8[:], rec8[:], mi8_sb[:])

            ita = [bigp.tile([128, NH], F32, name=f"ita{c}", tag="big") for c in range(4)]
            for h in range(HEADS):
                kc, ko = h // 2, (h % 2) * 64
                r0 = rowsp.tile([1, NH], BF16, name="r0", tag="r0")
                nc.sync.dma_start(r0[:], r8[h:h + 1, :])
                sps = psml.tile([64, NH], F32, name="sps", tag="s")
                nc.tensor.matmul(sps[:], lhsT=ones_row[0:1, 0:64], rhs=r0[:],
                                 start=True, stop=True)
                nc.vector.tensor_mul(ita[kc][ko:ko + 64, :], pv_sb[h][0:64, :], sps[:])

            # ---- phase C: transpose C0, U, CV ----
            c0T = [pers.tile([128, NH], BF16, name=f"c0T{j}") for j in range(8)]
            for jc in range(8):
                tps = pgen.tile([128, NH], F32, name="psw", tag="w")
                for ic in range(4):
                    nc.tensor.transpose(tps[:, ts(ic, 128)], c0[ic][:, ts(jc, 128)],
                                        ident[:])
                nc.vector.tensor_scalar_mul(c0T[jc][:], tps[:], cv[:, 8 + jc:9 + jc])

            # ---- fused CV + final: psum_out[ic] accumulates bias, then each
            # c2 chunk as soon as itf[c2] lands ----
            wout_sb = load_w(wout, "wout")
            ops = [pgen.tile([128, D], F32, name=f"ops{ic}", tag="w") for ic in range(4)]
            for ic in range(4):
                nc.tensor.matmul(ops[ic][:], lhsT=ones_row[0:1, 0:128], rhs=bout_row,
                                 start=True, stop=False)
            itf = [bigp.tile([128, NH], BF16, name=f"itf{c}", tag="big") for c in range(4)]
            for c2 in range(4):
                cvps = pacc.tile([128, NH], F32, name="cvps", tag="a")
                for jc in range(8):
                    nc.tensor.matmul(cvps[:], lhsT=v_pl[jc][:, ts(c2, 128)],
                                     rhs=c0T[jc][:], start=(jc == 0), stop=False)
                nc.tensor.matmul(cvps[:], lhsT=u_row[0:1, ts(c2, 128)], rhs=omi_row,
                                 start=False, stop=True)
                nc.vector.tensor_add(itf[c2][:], ita[c2][:], cvps[:])
                for ic in range(4):
                    nc.tensor.matmul(ops[ic][:], lhsT=itf[c2][:, ts(ic, 128)],
                                     rhs=wout_sb[c2][:], start=False, stop=(c2 == 3))
            for ic in range(4):
                osb = outp.tile([128, D], F32, name="osb", tag="osb")
                nc.vector.tensor_copy(osb[:], ops[ic][:])
                nc.sync.dma_start(out[ts(ic, 128), :], osb[:])

    nc.compile()
    return nc


def get_nc():
    if "nc" not in _CACHE:
        _CACHE["nc"] = _build_nc()
    return _CACHE["nc"]


def make_in_maps(x, mask, adjacency_mat, distance_mat, W_qkv, W_out, b_out):
    x = np.ascontiguousarray(np.asarray(x, np.float32))
    mask = np.asarray(mask)
    adjacency_mat = np.asarray(adjacency_mat, np.float32)
    distance_mat = np.asarray(distance_mat, np.float32)
    W_qkv = np.asarray(W_qkv, np.float32)
    W_out_b = np.ascontiguousarray(np.asarray(W_out, np.float32)).astype(BF)
    b_out = np.asarray(b_out, np.float32)

    W3 = W_qkv.reshape(D, HEADS, 3, DH)
    wq = np.ascontiguousarray(W3[:, :, 0, :].reshape(D, D)).astype(BF)
    wk = np.ascontiguousarray(W3[:, :, 1, :].reshape(D, D)).astype(BF)
    wv = np.ascontiguousarray(W3[:, :, 2, :].reshape(D, D)).astype(BF)

    xT = [np.ascontiguousarray(x[b].T).astype(BF) for b in range(B)]
    wv_f32 = W3[:, :, 2, :].reshape(D, D).astype(np.float32)
    u_host = [(LA / N) * (x[b].sum(0) @ wv_f32) for b in range(B)]

    in_maps = []
    for core in range(NCORES):
        b, half = core // 2, core % 2
        i0 = half * NH
        mj = mask[b].astype(np.float32)
        mi = mask[b, i0:i0 + NH].astype(np.float32)

        biasj = np.where(mj > 0, np.float32(np.log(LA)), np.float32(NEG))
        lnldmi = np.where(mi > 0, np.float32(np.log(LD)), np.float32(NEG))
        lgmi = (LG * mi).astype(np.float32)

        cvec = np.zeros((128, 24), np.float32)
        cvec[:, 0:8] = biasj.reshape(8, 128).T
        cvec[:, 8:16] = mj.reshape(8, 128).T
        cvec[:, 16:20] = lnldmi.reshape(4, 128).T
        cvec[:, 20:24] = lgmi.reshape(4, 128).T

        rvec = np.zeros((1, 1664), np.float32)
        rvec[0, 0:NH] = 1.0 - mi
        rvec[0, NH:NH + D] = b_out
        rvec[0, 2 * NH:2 * NH + 128] = 1.0
        rvec[0, 2 * NH + 128:2 * NH + 128 + D] = u_host[b]

        # mi8 also carries the exact correction for the bf16-rounded 1/LA
        # ones-column: computed softmax part = LA*P/((LA*(1/LA)_bf16)*sumP)
        corr = LA * INV_LA_BF
        mi8v = (np.tile(mi[None, :], (8, 1)) * corr).astype(np.float32)

        in_maps.append({
            "xT": xT[b],
            "xTq": np.ascontiguousarray(x[b, i0:i0 + NH, :].T).astype(BF),
            "wq": wq, "wk": wk, "wv": wv,
            "wout": W_out_b,
            "adj": np.ascontiguousarray(adjacency_mat[b, i0:i0 + NH, :]),
            "dist": np.ascontiguousarray(distance_mat[b, i0:i0 + NH, :]),
            "cvec": cvec,
            "rvec": rvec.astype(BF),
            "mi8": mi8v,
        })
    return in_maps


def kernel(x, mask, adjacency_mat, distance_mat, W_qkv, W_out, b_out):
    from concourse.bass_utils import run_bass_kernel_spmd

    nc = get_nc()
    in_maps = make_in_maps(x, mask, adjacency_mat, distance_mat, W_qkv, W_out, b_out)
    res = run_bass_kernel_spmd(nc, in_maps, core_ids=list(range(NCORES)))
    out_full = np.zeros((B, N, D), np.float32)
    for core in range(NCORES):
        b, half = core // 2, core % 2
        out_full[b, half * NH:(half + 1) * NH, :] = res.results[core]["out"]
    return out_full


# revision 14
# speedup vs baseline: 1.2356x; 1.2356x over previous
"""Trainium2 Bass kernel for nn_Attention_3315714753146 (gnn_message_passing).

out = (LA*softmax(mask(QK^T*scale)) + LG*adj_masked + LD*exp(-dist_masked)) @ V @ W_out + b_out

Sharding: 8 shards = (4 batches) x (2 query-row halves of 512 rows). Each core
computes its own 512 output rows from full K/V (computed on-device from x).
No collectives; gather on host.

Device algorithm per core (bf16 matmul operands, fp32 PSUM accumulation,
fp32 exp paths):
  qT = wq^T @ xT_own, kT = wk^T @ xT, v = xT^T @ wv (head-major weights)
  per head: dotsT[j,i] = kT_h chunks as lhsT against qT_h
            pT = exp(0.125*dotsT + biasj)   (biasj = ln(LA) | -1e30 -> folds LA + col mask)
            pv[65, i] = [v_h | 1/LA]^T @ pT (row 64 = softmax denominator)
            innerT_h = pv[0:64] * (maski/denom)  via PE outer-product broadcast
  C0 = (adj * LG*maski) + LD*maski*exp(-dist)  (fp32); transposed on PE, col-masked
  cvT = V^T @ C0T  (+ outer(LA/N * colsum(V), 1-maski) for invalid query rows)
  out = (innerT + cvT)^T @ W_out + b_out  (bias via K=1 ones outer into PSUM)
"""

import sys

for _p in ("/root/.axon_site", "/root/.axon_site/_ro/trn_rl_repo",
           "/root/.axon_site/_ro/pypackages"):
    if _p not in sys.path:
        sys.path.append(_p)

import numpy as np
import ml_dtypes

BF = ml_dtypes.bfloat16
HEADS, DH = 8, 64
B, N, D = 4, 1024, 512
NH = 512          # query rows per core
LA = LD = LG = 0.33
SCALE = DH ** -0.5
NEG = -1e30
NCORES = 8
# exact compensation for the bf16-rounded 1/LA ones-column in vaug
INV_LA_BF = float(np.float32(BF(1.0 / LA)))

_CACHE = {}


def _build_nc():
    import concourse.bass as bass
    import concourse.bacc as bacc
    import concourse.tile as tile
    from concourse import mybir
    from concourse.bass import ts
    from concourse.masks import make_identity

    F32 = mybir.dt.float32
    BF16 = mybir.dt.bfloat16
    AF = mybir.ActivationFunctionType
    OP = mybir.AluOpType

    nc = bacc.Bacc()
    xT = nc.declare_dram_parameter("xT", [D, N], BF16, isOutput=False)
    xTq = nc.declare_dram_parameter("xTq", [D, NH], BF16, isOutput=False)
    wq = nc.declare_dram_parameter("wq", [D, D], BF16, isOutput=False)
    wk = nc.declare_dram_parameter("wk", [D, D], BF16, isOutput=False)
    wv = nc.declare_dram_parameter("wv", [D, D], BF16, isOutput=False)
    wout = nc.declare_dram_parameter("wout", [D, D], BF16, isOutput=False)
    adj = nc.declare_dram_parameter("adj", [NH, N], F32, isOutput=False)
    dist = nc.declare_dram_parameter("dist", [NH, N], F32, isOutput=False)
    # cvec columns: [0:8]=biasj, [8:16]=maskj, [16:20]=lnldmi, [20:24]=lgmi
    cvec = nc.declare_dram_parameter("cvec", [128, 24], F32, isOutput=False)
    # rvec (bf16): [0:512]=1-maski, [512:1024]=b_out, [1024:1152]=ones,
    #              [1152:1664]=u_row (LA/N * colsum(V), host-computed)
    rvec = nc.declare_dram_parameter("rvec", [1, 1664], BF16, isOutput=False)
    mi8 = nc.declare_dram_parameter("mi8", [8, NH], F32, isOutput=False)
    out = nc.declare_dram_parameter("out", [NH, D], F32, isOutput=True)

    with tile.TileContext(nc) as tc:
        with (
            tc.tile_pool(name="const", bufs=1) as constp,
            tc.tile_pool(name="pers", bufs=1) as pers,
            tc.tile_pool(name="big", bufs=8) as bigp,
            tc.tile_pool(name="med", bufs=8) as medp,
            tc.tile_pool(name="wpool", bufs=8) as wpool,
            tc.tile_pool(name="stage", bufs=2) as stagep,
            tc.tile_pool(name="outp", bufs=2) as outp,
            tc.tile_pool(name="rows", bufs=2) as rowsp,
            tc.tile_pool(name="ppool", bufs=17) as ppool,
            tc.tile_pool(name="pgen", bufs=4, space="PSUM") as pgen,
            tc.tile_pool(name="pacc", bufs=2, space="PSUM") as pacc,
            tc.tile_pool(name="psml", bufs=2, space="PSUM") as psml,
        ):
            ident = constp.tile([128, 128], F32, name="ident")
            make_identity(nc, ident[:])
            onesb = constp.tile([128, 1], BF16, name="onesb")
            nc.gpsimd.memset(onesb[:], 1.0)
            cv = constp.tile([128, 24], F32, name="cv")
            nc.sync.dma_start(cv[:], cvec[:])
            rv = constp.tile([1, 1664], BF16, name="rv")
            nc.sync.dma_start(rv[:], rvec[:])
            omi_row = rv[0:1, 0:NH]
            bout_row = rv[0:1, NH:NH + D]
            ones_row = rv[0:1, 2 * NH:2 * NH + 128]
            u_row = rv[0:1, 2 * NH + 128:2 * NH + 128 + D]

            # ---- phase A: projections (one consolidated DMA per tensor,
            # q-path loads first so PE starts early) ----
            def load_w(w_dram, nm):
                t = wpool.tile([128, 4, D], BF16, name=f"{nm}_all", tag="w")
                nc.sync.dma_start(t[:], w_dram[:].rearrange("(c p) d -> p c d", p=128))
                return [t[:, c, :] for c in range(4)]

            wq_sb = load_w(wq, "wq")
            xtq_all = medp.tile([128, 4, NH], BF16, name="xtq_all", tag="medx")
            nc.sync.dma_start(xtq_all[:], xTq[:].rearrange("(c p) d -> p c d", p=128))
            xtq = [xtq_all[:, c, :] for c in range(4)]
            wk_sb = load_w(wk, "wk")
            xt_all = pers.tile([128, 4, N], BF16, name="xt_all")
            nc.sync.dma_start(xt_all[:], xT[:].rearrange("(c p) d -> p c d", p=128))
            xt = [xt_all[:, c, :] for c in range(4)]

            qt = []
            for oc in range(4):
                ps = pgen.tile([128, NH], F32, name="psw", tag="w")
                for cc in range(4):
                    nc.tensor.matmul(ps[:], lhsT=wq_sb[cc][:, ts(oc, 128)],
                                     rhs=xtq[cc][:], start=(cc == 0), stop=(cc == 3))
                t = pers.tile([128, NH], BF16, name=f"qt{oc}")
                nc.scalar.copy(t[:], ps[:])
                qt.append(t)

            kt = [bigp.tile([128, N], BF16, name=f"kt{oc}", tag="big") for oc in range(4)]
            for oc in range(4):
                for nn in range(2):
                    ps = pgen.tile([128, NH], F32, name="psw", tag="w")
                    for cc in range(4):
                        nc.tensor.matmul(ps[:], lhsT=wk_sb[cc][:, ts(oc, 128)],
                                         rhs=xt[cc][:, ts(nn, 512)],
                                         start=(cc == 0), stop=(cc == 3))
                    nc.scalar.copy(kt[oc][:, ts(nn, 512)], ps[:])

            wv_sb = load_w(wv, "wv")
            vaug = [pers.tile([128, HEADS * 65], BF16, name=f"vaug{j}") for j in range(8)]
            v_pl = [pers.tile([128, D], BF16, name=f"vpl{j}") for j in range(8)]
            for ncc in range(8):
                ps = pgen.tile([128, NH], F32, name="psw", tag="w")
                for cc in range(4):
                    nc.tensor.matmul(ps[:], lhsT=xt[cc][:, ts(ncc, 128)],
                                     rhs=wv_sb[cc][:], start=(cc == 0), stop=(cc == 3))
                nc.vector.tensor_copy(v_pl[ncc][:], ps[:])
                v3 = vaug[ncc][:].rearrange("p (h e) -> p h e", e=65)
                nc.vector.tensor_copy(v3[:, :, 0:64],
                                      v_pl[ncc][:].rearrange("p (h d) -> p h d", d=64))
                nc.gpsimd.memset(v3[:, :, 64:65], 1.0 / LA)

            # ---- phase C0 prep (fp32 exp path) ----
            c0 = [bigp.tile([128, N], F32, name=f"c0{ic}", tag="big") for ic in range(4)]
            for ic in range(4):
                adj_t = stagep.tile([128, N], F32, name="adj_t", tag="adj")
                nc.sync.dma_start(adj_t[:], adj[ts(ic, 128), :])
                dist_t = stagep.tile([128, N], F32, name="dist_t", tag="dist")
                nc.sync.dma_start(dist_t[:], dist[ts(ic, 128), :])
                e_t = stagep.tile([128, N], F32, name="e_t", tag="e")
                nc.scalar.activation(e_t[:], dist_t[:], AF.Exp,
                                     bias=cv[:, 16 + ic:17 + ic], scale=-1.0)
                nc.vector.scalar_tensor_tensor(out=c0[ic][:], in0=adj_t[:],
                                               scalar=cv[:, 20 + ic:21 + ic],
                                               in1=e_t[:], op0=OP.mult, op1=OP.add)

            # ---- phase B1: per-head dots -> exp -> PV; C0 transposes
            # interleaved to fill PE exp-shadow ----
            pv_sb = [medp.tile([65, NH], F32, name=f"pvsb{h}", tag="med") for h in range(HEADS)]
            denoms8 = pers.tile([8, NH], F32, name="denoms8")
            c0T = [pers.tile([128, NH], BF16, name=f"c0T{j}") for j in range(8)]
            for h in range(HEADS):
                kc, ko = h // 2, (h % 2) * 64
                pts = []
                for jc in range(8):
                    dps = pgen.tile([128, NH], F32, name="psw", tag="w")
                    nc.tensor.matmul(dps[:], lhsT=kt[kc][ko:ko + 64, ts(jc, 128)],
                                     rhs=qt[kc][ko:ko + 64, :], start=True, stop=True)
                    p_t = ppool.tile([128, NH], BF16, name="p_t", tag="p")
                    nc.scalar.activation(p_t[:], dps[:], AF.Exp,
                                         bias=cv[:, jc:jc + 1], scale=SCALE)
                    pts.append(p_t)
                pvps = pacc.tile([65, NH], F32, name="pvps", tag="a")
                for jc in range(8):
                    nc.tensor.matmul(pvps[:], lhsT=vaug[jc][:, h * 65:(h + 1) * 65],
                                     rhs=pts[jc][:], start=(jc == 0), stop=(jc == 7))
                nc.vector.tensor_copy(pv_sb[h][:], pvps[:])
                nc.sync.dma_start(denoms8[h:h + 1, :], pv_sb[h][64:65, :])
                jc = h
                tps = pgen.tile([128, NH], F32, name="psw", tag="w")
                for ic in range(4):
                    nc.tensor.transpose(tps[:, ts(ic, 128)], c0[ic][:, ts(jc, 128)],
                                        ident[:])
                nc.vector.tensor_scalar_mul(c0T[jc][:], tps[:], cv[:, 8 + jc:9 + jc])


            # ---- phase B2: normalize + mask_i via PE outer broadcast ----
            mi8_sb = constp.tile([8, NH], F32, name="mi8_sb")
            nc.sync.dma_start(mi8_sb[:], mi8[:])
            rec8 = pers.tile([8, NH], F32, name="rec8")
            nc.vector.reciprocal(rec8[:], denoms8[:])
            r8 = pers.tile([8, NH], BF16, name="r8")
            nc.vector.tensor_mul(r8[:], rec8[:], mi8_sb[:])

            ita = [bigp.tile([128, NH], F32, name=f"ita{c}", tag="big") for c in range(4)]
            for h in range(HEADS):
                kc, ko = h // 2, (h % 2) * 64
                r0 = rowsp.tile([1, NH], BF16, name="r0", tag="r0")
                nc.sync.dma_start(r0[:], r8[h:h + 1, :])
                sps = psml.tile([64, NH], F32, name="sps", tag="s")
                nc.tensor.matmul(sps[:], lhsT=ones_row[0:1, 0:64], rhs=r0[:],
                                 start=True, stop=True)
                nc.vector.tensor_mul(ita[kc][ko:ko + 64, :], pv_sb[h][0:64, :], sps[:])

            # ---- fused CV + final: psum_out[ic] accumulates bias, then each
            # c2 chunk as soon as itf[c2] lands ----
            wout_sb = load_w(wout, "wout")
            ops = [pgen.tile([128, D], F32, name=f"ops{ic}", tag="w") for ic in range(4)]
            for ic in range(4):
                nc.tensor.matmul(ops[ic][:], lhsT=ones_row[0:1, 0:128], rhs=bout_row,
                                 start=True, stop=False)
            itf = [bigp.tile([128, NH], BF16, name=f"itf{c}", tag="big") for c in range(4)]
            for c2 in range(4):
                cvps = pacc.tile([128, NH], F32, name="cvps", tag="a")
                for jc in range(8):
                    nc.tensor.matmul(cvps[:], lhsT=v_pl[jc][:, ts(c2, 128)],
                                     rhs=c0T[jc][:], start=(jc == 0), stop=False)
                nc.tensor.matmul(cvps[:], lhsT=u_row[0:1, ts(c2, 128)], rhs=omi_row,
                                 start=False, stop=True)
                nc.vector.tensor_add(itf[c2][:], ita[c2][:], cvps[:])
                for ic in range(4):
                    nc.tensor.matmul(ops[ic][:], lhsT=itf[c2][:, ts(ic, 128)],
                                     rhs=wout_sb[c2][:], start=False, stop=(c2 == 3))
            for ic in range(4):
                osb = outp.tile([128, D], F32, name="osb", tag="osb")
                nc.vector.tensor_copy(osb[:], ops[ic][:])
                nc.sync.dma_start(out[ts(ic, 128), :], osb[:])

    nc.compile()
    return nc


def get_nc():
    if "nc" not in _CACHE:
        _CACHE["nc"] = _build_nc()
    return _CACHE["nc"]


def make_in_maps(x, mask, adjacency_mat, distance_mat, W_qkv, W_out, b_out):
    x = np.ascontiguousarray(np.asarray(x, np.float32))
    mask = np.asarray(mask)
    adjacency_mat = np.asarray(adjacency_mat, np.float32)
    distance_mat = np.asarray(distance_mat, np.float32)
    W_qkv = np.asarray(W_qkv, np.float32)
    W_out_b = np.ascontiguousarray(np.asarray(W_out, np.float32)).astype(BF)
    b_out = np.asarray(b_out, np.float32)

    W3 = W_qkv.reshape(D, HEADS, 3, DH)
    wq = np.ascontiguousarray(W3[:, :, 0, :].reshape(D, D)).astype(BF)
    wk = np.ascontiguousarray(W3[:, :, 1, :].reshape(D, D)).astype(BF)
    wv = np.ascontiguousarray(W3[:, :, 2, :].reshape(D, D)).astype(BF)

    xT = [np.ascontiguousarray(x[b].T).astype(BF) for b in range(B)]
    wv_f32 = W3[:, :, 2, :].reshape(D, D).astype(np.float32)
    u_host = [(LA / N) * (x[b].sum(0) @ wv_f32) for b in range(B)]

    in_maps = []
    for core in range(NCORES):
        b, half = core // 2, core % 2
        i0 = half * NH
        mj = mask[b].astype(np.float32)
        mi = mask[b, i0:i0 + NH].astype(np.float32)

        biasj = np.where(mj > 0, np.float32(np.log(LA)), np.float32(NEG))
        lnldmi = np.where(mi > 0, np.float32(np.log(LD)), np.float32(NEG))
        lgmi = (LG * mi).astype(np.float32)

        cvec = np.zeros((128, 24), np.float32)
        cvec[:, 0:8] = biasj.reshape(8, 128).T
        cvec[:, 8:16] = mj.reshape(8, 128).T
        cvec[:, 16:20] = lnldmi.reshape(4, 128).T
        cvec[:, 20:24] = lgmi.reshape(4, 128).T

        rvec = np.zeros((1, 1664), np.float32)
        rvec[0, 0:NH] = 1.0 - mi
        rvec[0, NH:NH + D] = b_out
        rvec[0, 2 * NH:2 * NH + 128] = 1.0
        rvec[0, 2 * NH + 128:2 * NH + 128 + D] = u_host[b]

        # mi8 also carries the exact correction for the bf16-rounded 1/LA
        # ones-column: computed softmax part = LA*P/((LA*(1/LA)_bf16)*sumP)
        corr = LA * INV_LA_BF
        mi8v = (np.tile(mi[None, :], (8, 1)) * corr).astype(np.float32)

        in_maps.append({
            "xT": xT[b],
            "xTq": np.ascontiguousarray(x[b, i0:i0 + NH, :].T).astype(BF),
            "wq": wq, "wk": wk, "wv": wv,
            "wout": W_out_b,
            "adj": np.ascontiguousarray(adjacency_mat[b, i0:i0 + NH, :]),
            "dist": np.ascontiguousarray(distance_mat[b, i0:i0 + NH, :]),
            "cvec": cvec,
            "rvec": rvec.astype(BF),
            "mi8": mi8v,
        })
    return in_maps


def kernel(x, mask, adjacency_mat, distance_mat, W_qkv, W_out, b_out):
    from concourse.bass_utils import run_bass_kernel_spmd

    nc = get_nc()
    in_maps = make_in_maps(x, mask, adjacency_mat, distance_mat, W_qkv, W_out, b_out)
    res = run_bass_kernel_spmd(nc, in_maps, core_ids=list(range(NCORES)))
    out_full = np.zeros((B, N, D), np.float32)
    for core in range(NCORES):
        b, half = core // 2, core % 2
        out_full[b, half * NH:(half + 1) * NH, :] = res.results[core]["out"]
    return out_full


# revision 17
# speedup vs baseline: 1.2450x; 1.0076x over previous
"""Trainium2 Bass kernel for nn_Attention_3315714753146 (gnn_message_passing).

out = (LA*softmax(mask(QK^T*scale)) + LG*adj_masked + LD*exp(-dist_masked)) @ V @ W_out + b_out

Sharding: 8 shards = (4 batches) x (2 query-row halves of 512 rows). Each core
computes its own 512 output rows from full K/V (computed on-device from x).
No collectives; gather on host.

Device algorithm per core (bf16 matmul operands, fp32 PSUM accumulation,
fp32 exp paths):
  qT = wq^T @ xT_own, kT = wk^T @ xT, v = xT^T @ wv (head-major weights)
  per head: dotsT[j,i] = kT_h chunks as lhsT against qT_h
            pT = exp(0.125*dotsT + biasj)   (biasj = ln(LA) | -1e30 -> folds LA + col mask)
            pv[65, i] = [v_h | 1/LA]^T @ pT (row 64 = softmax denominator)
            innerT_h = pv[0:64] * (maski/denom)  via PE outer-product broadcast
  C0 = (adj * LG*maski) + LD*maski*exp(-dist)  (fp32); transposed on PE, col-masked
  cvT = V^T @ C0T  (+ outer(LA/N * colsum(V), 1-maski) for invalid query rows)
  out = (innerT + cvT)^T @ W_out + b_out  (bias via K=1 ones outer into PSUM)
"""

import sys

for _p in ("/root/.axon_site", "/root/.axon_site/_ro/trn_rl_repo",
           "/root/.axon_site/_ro/pypackages"):
    if _p not in sys.path:
        sys.path.append(_p)

import numpy as np
import ml_dtypes

BF = ml_dtypes.bfloat16
HEADS, DH = 8, 64
B, N, D = 4, 1024, 512
NH = 512          # query rows per core
LA = LD = LG = 0.33
SCALE = DH ** -0.5
NEG = -1e30
NCORES = 8
# exact compensation for the bf16-rounded 1/LA ones-column in vaug
INV_LA_BF = float(np.float32(BF(1.0 / LA)))

_CACHE = {}


def _build_nc():
    import concourse.bass as bass
    import concourse.bacc as bacc
    import concourse.tile as tile
    from concourse import mybir
    from concourse.bass import ts
    from concourse.masks import make_identity

    F32 = mybir.dt.float32
    BF16 = mybir.dt.bfloat16
    AF = mybir.ActivationFunctionType
    OP = mybir.AluOpType

    nc = bacc.Bacc()
    xT = nc.declare_dram_parameter("xT", [D, N], BF16, isOutput=False)
    xTq = nc.declare_dram_parameter("xTq", [D, NH], BF16, isOutput=False)
    wq = nc.declare_dram_parameter("wq", [D, D], BF16, isOutput=False)
    wk = nc.declare_dram_parameter("wk", [D, D], BF16, isOutput=False)
    wv = nc.declare_dram_parameter("wv", [D, D], BF16, isOutput=False)
    wout = nc.declare_dram_parameter("wout", [D, D], BF16, isOutput=False)
    adj = nc.declare_dram_parameter("adj", [NH, N], F32, isOutput=False)
    dist = nc.declare_dram_parameter("dist", [NH, N], F32, isOutput=False)
    # cvec columns: [0:8]=biasj, [8:16]=maskj, [16:20]=lnldmi, [20:24]=lgmi
    cvec = nc.declare_dram_parameter("cvec", [128, 24], F32, isOutput=False)
    # rvec (bf16): [0:512]=1-maski, [512:1024]=b_out, [1024:1152]=ones,
    #              [1152:1664]=u_row (LA/N * colsum(V), host-computed)
    rvec = nc.declare_dram_parameter("rvec", [1, 1664], BF16, isOutput=False)
    mi8 = nc.declare_dram_parameter("mi8", [8, NH], F32, isOutput=False)
    out = nc.declare_dram_parameter("out", [NH, D], F32, isOutput=True)

    with tile.TileContext(nc) as tc:
        with (
            tc.tile_pool(name="const", bufs=1) as constp,
            tc.tile_pool(name="pers", bufs=1) as pers,
            tc.tile_pool(name="big", bufs=8) as bigp,
            tc.tile_pool(name="med", bufs=8) as medp,
            tc.tile_pool(name="wpool", bufs=8) as wpool,
            tc.tile_pool(name="stage", bufs=2) as stagep,
            tc.tile_pool(name="outp", bufs=2) as outp,
            tc.tile_pool(name="rows", bufs=2) as rowsp,
            tc.tile_pool(name="ppool", bufs=17) as ppool,
            tc.tile_pool(name="pgen", bufs=4, space="PSUM") as pgen,
            tc.tile_pool(name="pacc", bufs=2, space="PSUM") as pacc,
            tc.tile_pool(name="psml", bufs=2, space="PSUM") as psml,
        ):
            ident = constp.tile([128, 128], F32, name="ident")
            make_identity(nc, ident[:])
            onesb = constp.tile([128, 1], BF16, name="onesb")
            nc.gpsimd.memset(onesb[:], 1.0)
            # ---- phase A: projections (one consolidated DMA per tensor,
            # q-path loads first so PE starts early) ----
            def load_w(w_dram, nm):
                t = wpool.tile([128, 4, D], BF16, name=f"{nm}_all", tag="w")
                nc.sync.dma_start(t[:], w_dram[:].rearrange("(c p) d -> p c d", p=128))
                return [t[:, c, :] for c in range(4)]

            wq_sb = load_w(wq, "wq")
            xtq_all = medp.tile([128, 4, NH], BF16, name="xtq_all", tag="medx")
            nc.sync.dma_start(xtq_all[:], xTq[:].rearrange("(c p) d -> p c d", p=128))
            xtq = [xtq_all[:, c, :] for c in range(4)]
            wk_sb = load_w(wk, "wk")
            xt_all = pers.tile([128, 4, N], BF16, name="xt_all")
            nc.sync.dma_start(xt_all[:], xT[:].rearrange("(c p) d -> p c d", p=128))
            xt = [xt_all[:, c, :] for c in range(4)]
            cv = constp.tile([128, 24], F32, name="cv")
            nc.sync.dma_start(cv[:], cvec[:])
            rv = constp.tile([1, 1664], BF16, name="rv")
            nc.sync.dma_start(rv[:], rvec[:])
            omi_row = rv[0:1, 0:NH]
            bout_row = rv[0:1, NH:NH + D]
            ones_row = rv[0:1, 2 * NH:2 * NH + 128]
            u_row = rv[0:1, 2 * NH + 128:2 * NH + 128 + D]

            qt = []
            for oc in range(4):
                ps = pgen.tile([128, NH], F32, name="psw", tag="w")
                for cc in range(4):
                    nc.tensor.matmul(ps[:], lhsT=wq_sb[cc][:, ts(oc, 128)],
                                     rhs=xtq[cc][:], start=(cc == 0), stop=(cc == 3))
                t = pers.tile([128, NH], BF16, name=f"qt{oc}")
                nc.scalar.copy(t[:], ps[:])
                qt.append(t)

            kt = [bigp.tile([128, N], BF16, name=f"kt{oc}", tag="big") for oc in range(4)]
            for oc in range(4):
                for nn in range(2):
                    ps = pgen.tile([128, NH], F32, name="psw", tag="w")
                    for cc in range(4):
                        nc.tensor.matmul(ps[:], lhsT=wk_sb[cc][:, ts(oc, 128)],
                                         rhs=xt[cc][:, ts(nn, 512)],
                                         start=(cc == 0), stop=(cc == 3))
                    nc.scalar.copy(kt[oc][:, ts(nn, 512)], ps[:])

            wv_sb = load_w(wv, "wv")
            vaug = [pers.tile([128, HEADS * 65], BF16, name=f"vaug{j}") for j in range(8)]
            v_pl = [pers.tile([128, D], BF16, name=f"vpl{j}") for j in range(8)]
            for ncc in range(8):
                ps = pgen.tile([128, NH], F32, name="psw", tag="w")
                for cc in range(4):
                    nc.tensor.matmul(ps[:], lhsT=xt[cc][:, ts(ncc, 128)],
                                     rhs=wv_sb[cc][:], start=(cc == 0), stop=(cc == 3))
                nc.vector.tensor_copy(v_pl[ncc][:], ps[:])
                v3 = vaug[ncc][:].rearrange("p (h e) -> p h e", e=65)
                nc.vector.tensor_copy(v3[:, :, 0:64],
                                      v_pl[ncc][:].rearrange("p (h d) -> p h d", d=64))
                nc.gpsimd.memset(v3[:, :, 64:65], 1.0 / LA)

            # ---- phase C0 prep (fp32 exp path) ----
            c0 = [bigp.tile([128, N], F32, name=f"c0{ic}", tag="big") for ic in range(4)]
            for ic in range(4):
                adj_t = stagep.tile([128, N], F32, name="adj_t", tag="adj")
                nc.gpsimd.dma_start(adj_t[:], adj[ts(ic, 128), :])
                dist_t = stagep.tile([128, N], F32, name="dist_t", tag="dist")
                nc.gpsimd.dma_start(dist_t[:], dist[ts(ic, 128), :])
                e_t = stagep.tile([128, N], F32, name="e_t", tag="e")
                nc.scalar.activation(e_t[:], dist_t[:], AF.Exp,
                                     bias=cv[:, 16 + ic:17 + ic], scale=-1.0)
                nc.vector.scalar_tensor_tensor(out=c0[ic][:], in0=adj_t[:],
                                               scalar=cv[:, 20 + ic:21 + ic],
                                               in1=e_t[:], op0=OP.mult, op1=OP.add)

            # ---- phase B1: per-head dots -> exp -> PV; C0 transposes
            # interleaved to fill PE exp-shadow ----
            pv_sb = [medp.tile([65, NH], F32, name=f"pvsb{h}", tag="med") for h in range(HEADS)]
            denoms8 = pers.tile([8, NH], F32, name="denoms8")
            c0T = [pers.tile([128, NH], BF16, name=f"c0T{j}") for j in range(8)]
            for hp in range(4):
                kc = hp
                pts = {0: [], 1: []}
                for jc in range(8):
                    for par in range(2):
                        ko = par * 64
                        dps = pgen.tile([128, NH], F32, name="psw", tag="w")
                        nc.tensor.matmul(dps[:], lhsT=kt[kc][ko:ko + 64, ts(jc, 128)],
                                         rhs=qt[kc][ko:ko + 64, :], start=True,
                                         stop=True, tile_position=(ko, 0))
                        p_t = ppool.tile([128, NH], BF16, name="p_t", tag="p")
                        nc.scalar.activation(p_t[:], dps[:], AF.Exp,
                                             bias=cv[:, jc:jc + 1], scale=SCALE)
                        pts[par].append(p_t)
                for par in range(2):
                    h = 2 * hp + par
                    pvps = pacc.tile([65, NH], F32, name="pvps", tag="a")
                    for jc in range(8):
                        nc.tensor.matmul(pvps[:], lhsT=vaug[jc][:, h * 65:(h + 1) * 65],
                                         rhs=pts[par][jc][:], start=(jc == 0), stop=(jc == 7))
                    nc.vector.tensor_copy(pv_sb[h][:], pvps[:])
                    nc.gpsimd.dma_start(denoms8[h:h + 1, :], pv_sb[h][64:65, :])
                    jc = 2 * hp + par
                    tps = pgen.tile([128, NH], F32, name="psw", tag="w")
                    for ic in range(4):
                        nc.tensor.transpose(tps[:, ts(ic, 128)], c0[ic][:, ts(jc, 128)],
                                            ident[:])
                    nc.vector.tensor_scalar_mul(c0T[jc][:], tps[:], cv[:, 8 + jc:9 + jc])


            # ---- phase B2: normalize + mask_i via PE outer broadcast ----
            mi8_sb = constp.tile([8, NH], F32, name="mi8_sb")
            nc.gpsimd.dma_start(mi8_sb[:], mi8[:])
            rec8 = pers.tile([8, NH], F32, name="rec8")
            nc.vector.reciprocal(rec8[:], denoms8[:])
            r8 = pers.tile([8, NH], BF16, name="r8")
            nc.vector.tensor_mul(r8[:], rec8[:], mi8_sb[:])

            ita = [bigp.tile([128, NH], F32, name=f"ita{c}", tag="big") for c in range(4)]
            for h in range(HEADS):
                kc, ko = h // 2, (h % 2) * 64
                r0 = rowsp.tile([1, NH], BF16, name="r0", tag="r0")
                nc.gpsimd.dma_start(r0[:], r8[h:h + 1, :])
                sps = psml.tile([64, NH], F32, name="sps", tag="s")
                nc.tensor.matmul(sps[:], lhsT=ones_row[0:1, 0:64], rhs=r0[:],
                                 start=True, stop=True)
                nc.vector.tensor_mul(ita[kc][ko:ko + 64, :], pv_sb[h][0:64, :], sps[:])

            # ---- fused CV + final: psum_out[ic] accumulates bias, then each
            # c2 chunk as soon as itf[c2] lands ----
            wout_sb = load_w(wout, "wout")
            ops = [pgen.tile([128, D], F32, name=f"ops{ic}", tag="w") for ic in range(4)]
            for ic in range(4):
                nc.tensor.matmul(ops[ic][:], lhsT=ones_row[0:1, 0:128], rhs=bout_row,
                                 start=True, stop=False)
            itf = [bigp.tile([128, NH], BF16, name=f"itf{c}", tag="big") for c in range(4)]
            for c2 in range(4):
                cvps = pacc.tile([128, NH], F32, name="cvps", tag="a")
                for jc in range(8):
                    nc.tensor.matmul(cvps[:], lhsT=v_pl[jc][:, ts(c2, 128)],
                                     rhs=c0T[jc][:], start=(jc == 0), stop=False)
                nc.tensor.matmul(cvps[:], lhsT=u_row[0:1, ts(c2, 128)], rhs=omi_row,
                                 start=False, stop=True)
                nc.vector.tensor_add(itf[c2][:], ita[c2][:], cvps[:])
                for ic in range(4):
                    nc.tensor.matmul(ops[ic][:], lhsT=itf[c2][:, ts(ic, 128)],
                                     rhs=wout_sb[c2][:], start=False, stop=(c2 == 3))
            for ic in range(4):
                osb = outp.tile([128, D], F32, name="osb", tag="osb")
                nc.vector.tensor_copy(osb[:], ops[ic][:])
                nc.sync.dma_start(out[ts(ic, 128), :], osb[:])

    nc.compile()
    return nc


def get_nc():
    if "nc" not in _CACHE:
        _CACHE["nc"] = _build_nc()
    return _CACHE["nc"]


def make_in_maps(x, mask, adjacency_mat, distance_mat, W_qkv, W_out, b_out):
    x = np.ascontiguousarray(np.asarray(x, np.float32))
    mask = np.asarray(mask)
    adjacency_mat = np.asarray(adjacency_mat, np.float32)
    distance_mat = np.asarray(distance_mat, np.float32)
    W_qkv = np.asarray(W_qkv, np.float32)
    W_out_b = np.ascontiguousarray(np.asarray(W_out, np.float32)).astype(BF)
    b_out = np.asarray(b_out, np.float32)

    W3 = W_qkv.reshape(D, HEADS, 3, DH)
    wq = np.ascontiguousarray(W3[:, :, 0, :].reshape(D, D)).astype(BF)
    wk = np.ascontiguousarray(W3[:, :, 1, :].reshape(D, D)).astype(BF)
    wv = np.ascontiguousarray(W3[:, :, 2, :].reshape(D, D)).astype(BF)

    xT = [np.ascontiguousarray(x[b].T).astype(BF) for b in range(B)]
    wv_f32 = W3[:, :, 2, :].reshape(D, D).astype(np.float32)
    u_host = [(LA / N) * (x[b].sum(0) @ wv_f32) for b in range(B)]

    in_maps = []
    for core in range(NCORES):
        b, half = core // 2, core % 2
        i0 = half * NH
        mj = mask[b].astype(np.float32)
        mi = mask[b, i0:i0 + NH].astype(np.float32)

        biasj = np.where(mj > 0, np.float32(np.log(LA)), np.float32(NEG))
        lnldmi = np.where(mi > 0, np.float32(np.log(LD)), np.float32(NEG))
        lgmi = (LG * mi).astype(np.float32)

        cvec = np.zeros((128, 24), np.float32)
        cvec[:, 0:8] = biasj.reshape(8, 128).T
        cvec[:, 8:16] = mj.reshape(8, 128).T
        cvec[:, 16:20] = lnldmi.reshape(4, 128).T
        cvec[:, 20:24] = lgmi.reshape(4, 128).T

        rvec = np.zeros((1, 1664), np.float32)
        rvec[0, 0:NH] = 1.0 - mi
        rvec[0, NH:NH + D] = b_out
        rvec[0, 2 * NH:2 * NH + 128] = 1.0
        rvec[0, 2 * NH + 128:2 * NH + 128 + D] = u_host[b]

        # mi8 also carries the exact correction for the bf16-rounded 1/LA
        # ones-column: computed softmax part = LA*P/((LA*(1/LA)_bf16)*sumP)
        corr = LA * INV_LA_BF
        mi8v = (np.tile(mi[None, :], (8, 1)) * corr).astype(np.float32)

        in_maps.append({
            "xT": xT[b],
            "xTq": np.ascontiguousarray(x[b, i0:i0 + NH, :].T).astype(BF),
            "wq": wq, "wk": wk, "wv": wv,
            "wout": W_out_b,
            "adj": np.ascontiguousarray(adjacency_mat[b, i0:i0 + NH, :]),
            "dist": np.ascontiguousarray(distance_mat[b, i0:i0 + NH, :]),
            "cvec": cvec,
            "rvec": rvec.astype(BF),
            "mi8": mi8v,
        })
    return in_maps


def kernel(x, mask, adjacency_mat, distance_mat, W_qkv, W_out, b_out):
    from concourse.bass_utils import run_bass_kernel_spmd

    nc = get_nc()
    in_maps = make_in_maps(x, mask, adjacency_mat, distance_mat, W_qkv, W_out, b_out)
    res = run_bass_kernel_spmd(nc, in_maps, core_ids=list(range(NCORES)))
    out_full = np.zeros((B, N, D), np.float32)
    for core in range(NCORES):
        b, half = core // 2, core % 2
        out_full[b, half * NH:(half + 1) * NH, :] = res.results[core]["out"]
    return out_full


# revision 18
# speedup vs baseline: 1.3110x; 1.0529x over previous
"""Trainium2 Bass kernel for nn_Attention_3315714753146 (gnn_message_passing).

out = (LA*softmax(mask(QK^T*scale)) + LG*adj_masked + LD*exp(-dist_masked)) @ V @ W_out + b_out

Sharding: 8 shards = (4 batches) x (2 query-row halves of 512 rows). Each core
computes its own 512 output rows from full K/V (computed on-device from x).
No collectives; gather on host.

Device algorithm per core (bf16 matmul operands, fp32 PSUM accumulation,
fp32 exp paths):
  qT = wq^T @ xT_own, kT = wk^T @ xT, v = xT^T @ wv (head-major weights)
  per head: dotsT[j,i] = kT_h chunks as lhsT against qT_h
            pT = exp(0.125*dotsT + biasj)   (biasj = ln(LA) | -1e30 -> folds LA + col mask)
            pv[65, i] = [v_h | 1/LA]^T @ pT (row 64 = softmax denominator)
            innerT_h = pv[0:64] * (maski/denom)  via PE outer-product broadcast
  C0 = (adj * LG*maski) + LD*maski*exp(-dist)  (fp32); transposed on PE, col-masked
  cvT = V^T @ C0T  (+ outer(LA/N * colsum(V), 1-maski) for invalid query rows)
  out = (innerT + cvT)^T @ W_out + b_out  (bias via K=1 ones outer into PSUM)
"""

import sys

for _p in ("/root/.axon_site", "/root/.axon_site/_ro/trn_rl_repo",
           "/root/.axon_site/_ro/pypackages"):
    if _p not in sys.path:
        sys.path.append(_p)

import numpy as np
import ml_dtypes

BF = ml_dtypes.bfloat16
HEADS, DH = 8, 64
B, N, D = 4, 1024, 512
NH = 512          # query rows per core
LA = LD = LG = 0.33
SCALE = DH ** -0.5
NEG = -1e30
NCORES = 8
# exact compensation for the bf16-rounded 1/LA ones-column in vaug
INV_LA_BF = float(np.float32(BF(1.0 / LA)))

_CACHE = {}


def _build_nc():
    import concourse.bass as bass
    import concourse.bacc as bacc
    import concourse.tile as tile
    from concourse import mybir
    from concourse.bass import ts
    from concourse.masks import make_identity

    F32 = mybir.dt.float32
    BF16 = mybir.dt.bfloat16
    AF = mybir.ActivationFunctionType
    OP = mybir.AluOpType

    nc = bacc.Bacc()
    xT = nc.declare_dram_parameter("xT", [D, N], BF16, isOutput=False)
    xTq = nc.declare_dram_parameter("xTq", [D, NH], BF16, isOutput=False)
    wq = nc.declare_dram_parameter("wq", [D, D], BF16, isOutput=False)
    wk = nc.declare_dram_parameter("wk", [D, D], BF16, isOutput=False)
    wv = nc.declare_dram_parameter("wv", [D, D], BF16, isOutput=False)
    wout = nc.declare_dram_parameter("wout", [D, D], BF16, isOutput=False)
    adj = nc.declare_dram_parameter("adj", [NH, N], F32, isOutput=False)
    dist = nc.declare_dram_parameter("dist", [NH, N], F32, isOutput=False)
    # cvec columns: [0:8]=biasj, [8:16]=maskj, [16:20]=lnldmi, [20:24]=lgmi
    cvec = nc.declare_dram_parameter("cvec", [128, 24], F32, isOutput=False)
    # rvec (bf16): [0:512]=1-maski, [512:1024]=b_out, [1024:1152]=ones,
    #              [1152:1664]=u_row (LA/N * colsum(V), host-computed)
    rvec = nc.declare_dram_parameter("rvec", [1, 1664], BF16, isOutput=False)
    mi8 = nc.declare_dram_parameter("mi8", [8, NH], F32, isOutput=False)
    out = nc.declare_dram_parameter("out", [NH, D], F32, isOutput=True)

    with tile.TileContext(nc) as tc:
        with (
            tc.tile_pool(name="const", bufs=1) as constp,
            tc.tile_pool(name="pers", bufs=1) as pers,
            tc.tile_pool(name="big", bufs=8) as bigp,
            tc.tile_pool(name="med", bufs=8) as medp,
            tc.tile_pool(name="wpool", bufs=8) as wpool,
            tc.tile_pool(name="stage", bufs=2) as stagep,
            tc.tile_pool(name="outp", bufs=2) as outp,
            tc.tile_pool(name="rows", bufs=2) as rowsp,
            tc.tile_pool(name="ppool", bufs=17) as ppool,
            tc.tile_pool(name="pgen", bufs=4, space="PSUM") as pgen,
            tc.tile_pool(name="pacc", bufs=2, space="PSUM") as pacc,
            tc.tile_pool(name="psml", bufs=2, space="PSUM") as psml,
        ):
            ident = constp.tile([128, 128], F32, name="ident")
            make_identity(nc, ident[:])
            onesb = constp.tile([128, 1], BF16, name="onesb")
            nc.gpsimd.memset(onesb[:], 1.0)
            # ---- phase A: projections (one consolidated DMA per tensor,
            # q-path loads first so PE starts early) ----
            def load_w(w_dram, nm):
                t = wpool.tile([128, 4, D], BF16, name=f"{nm}_all", tag="w")
                nc.sync.dma_start(t[:], w_dram[:].rearrange("(c p) d -> p c d", p=128))
                return [t[:, c, :] for c in range(4)]

            wq_sb = load_w(wq, "wq")
            xtq_all = medp.tile([128, 4, NH], BF16, name="xtq_all", tag="medx")
            nc.sync.dma_start(xtq_all[:], xTq[:].rearrange("(c p) d -> p c d", p=128))
            xtq = [xtq_all[:, c, :] for c in range(4)]
            wk_sb = load_w(wk, "wk")
            xt_all = pers.tile([128, 4, N], BF16, name="xt_all")
            nc.sync.dma_start(xt_all[:], xT[:].rearrange("(c p) d -> p c d", p=128))
            xt = [xt_all[:, c, :] for c in range(4)]
            cv = constp.tile([128, 24], F32, name="cv")
            nc.sync.dma_start(cv[:], cvec[:])
            rv = constp.tile([1, 1664], BF16, name="rv")
            nc.sync.dma_start(rv[:], rvec[:])
            omi_row = rv[0:1, 0:NH]
            bout_row = rv[0:1, NH:NH + D]
            ones_row = rv[0:1, 2 * NH:2 * NH + 128]
            u_row = rv[0:1, 2 * NH + 128:2 * NH + 128 + D]

            qt = []
            for oc in range(4):
                ps = pgen.tile([128, NH], F32, name="psw", tag="w")
                for cc in range(4):
                    nc.tensor.matmul(ps[:], lhsT=wq_sb[cc][:, ts(oc, 128)],
                                     rhs=xtq[cc][:], start=(cc == 0), stop=(cc == 3))
                t = pers.tile([128, NH], BF16, name=f"qt{oc}")
                nc.scalar.copy(t[:], ps[:])
                qt.append(t)

            kt = [bigp.tile([128, N], BF16, name=f"kt{oc}", tag="big") for oc in range(4)]
            for oc in range(4):
                for nn in range(2):
                    ps = pgen.tile([128, NH], F32, name="psw", tag="w")
                    for cc in range(4):
                        nc.tensor.matmul(ps[:], lhsT=wk_sb[cc][:, ts(oc, 128)],
                                         rhs=xt[cc][:, ts(nn, 512)],
                                         start=(cc == 0), stop=(cc == 3))
                    nc.scalar.copy(kt[oc][:, ts(nn, 512)], ps[:])

            wv_sb = load_w(wv, "wv")
            vaug = [pers.tile([128, HEADS * 65], BF16, name=f"vaug{j}") for j in range(8)]
            v_pl = [pers.tile([128, D], BF16, name=f"vpl{j}") for j in range(8)]
            for ncc in range(8):
                ps = pgen.tile([128, NH], F32, name="psw", tag="w")
                for cc in range(4):
                    nc.tensor.matmul(ps[:], lhsT=xt[cc][:, ts(ncc, 128)],
                                     rhs=wv_sb[cc][:], start=(cc == 0), stop=(cc == 3))
                nc.vector.tensor_copy(v_pl[ncc][:], ps[:])
                v3 = vaug[ncc][:].rearrange("p (h e) -> p h e", e=65)
                nc.vector.tensor_copy(v3[:, :, 0:64],
                                      v_pl[ncc][:].rearrange("p (h d) -> p h d", d=64))
                nc.gpsimd.memset(v3[:, :, 64:65], 1.0 / LA)

            # ---- phase C0 prep (fp32 exp path) ----
            c0 = [bigp.tile([128, N], F32, name=f"c0{ic}", tag="big") for ic in range(4)]
            for ic in range(4):
                adj_t = stagep.tile([128, N], F32, name="adj_t", tag="adj")
                nc.gpsimd.dma_start(adj_t[:], adj[ts(ic, 128), :])
                dist_t = stagep.tile([128, N], F32, name="dist_t", tag="dist")
                nc.gpsimd.dma_start(dist_t[:], dist[ts(ic, 128), :])
                e_t = stagep.tile([128, N], F32, name="e_t", tag="e")
                nc.scalar.activation(e_t[:], dist_t[:], AF.Exp,
                                     bias=cv[:, 16 + ic:17 + ic], scale=-1.0)
                nc.vector.scalar_tensor_tensor(out=c0[ic][:], in0=adj_t[:],
                                               scalar=cv[:, 20 + ic:21 + ic],
                                               in1=e_t[:], op0=OP.mult, op1=OP.add)

            # ---- phase B1: per-head dots -> exp -> PV; C0 transposes
            # interleaved to fill PE exp-shadow ----
            pv_sb = [medp.tile([65, NH], F32, name=f"pvsb{h}", tag="med") for h in range(HEADS)]
            denoms8 = pers.tile([8, NH], F32, name="denoms8")
            c0T = [pers.tile([128, NH], BF16, name=f"c0T{j}") for j in range(8)]
            for hp in range(4):
                kc = hp
                pts = {0: [], 1: []}
                for jc in range(8):
                    for par in range(2):
                        ko = par * 64
                        dps = pgen.tile([128, NH], F32, name="psw", tag="w")
                        nc.tensor.matmul(dps[:], lhsT=kt[kc][ko:ko + 64, ts(jc, 128)],
                                         rhs=qt[kc][ko:ko + 64, :], start=True,
                                         stop=True, tile_position=(ko, 0))
                        p_t = ppool.tile([128, NH], BF16, name="p_t", tag="p")
                        nc.scalar.activation(p_t[:], dps[:], AF.Exp,
                                             bias=cv[:, jc:jc + 1], scale=SCALE)
                        pts[par].append(p_t)
                for par in range(2):
                    h = 2 * hp + par
                    pvps = pacc.tile([65, NH], F32, name="pvps", tag="a")
                    for jc in range(8):
                        nc.tensor.matmul(pvps[:], lhsT=vaug[jc][:, h * 65:(h + 1) * 65],
                                         rhs=pts[par][jc][:], start=(jc == 0), stop=(jc == 7))
                    nc.vector.tensor_copy(pv_sb[h][:], pvps[:])
                    nc.gpsimd.dma_start(denoms8[h:h + 1, :], pv_sb[h][64:65, :])
                    jc = 2 * hp + par
                    tps = pgen.tile([128, NH], F32, name="psw", tag="w")
                    for ic in range(4):
                        nc.tensor.transpose(tps[:, ts(ic, 128)], c0[ic][:, ts(jc, 128)],
                                            ident[:])
                    nc.vector.tensor_scalar_mul(c0T[jc][:], tps[:], cv[:, 8 + jc:9 + jc])


            # ---- CV accumulation immediately after B1 (fills the reciprocal
            # window); ops groups spread over pacc+psml so all 4 finals pipeline ----
            wout_sb = load_w(wout, "wout")
            cvs = []
            for c2 in range(4):
                cvps = pgen.tile([128, NH], F32, name="cvps", tag="w")
                for jc in range(8):
                    nc.tensor.matmul(cvps[:], lhsT=v_pl[jc][:, ts(c2, 128)],
                                     rhs=c0T[jc][:], start=(jc == 0), stop=False)
                nc.tensor.matmul(cvps[:], lhsT=u_row[0:1, ts(c2, 128)], rhs=omi_row,
                                 start=False, stop=True)
                cvs.append(cvps)

            # ---- phase B2: normalize + mask_i via PE outer broadcast ----
            mi8_sb = constp.tile([8, NH], F32, name="mi8_sb")
            nc.gpsimd.dma_start(mi8_sb[:], mi8[:])
            rec8 = pers.tile([8, NH], F32, name="rec8")
            nc.vector.reciprocal(rec8[:], denoms8[:])
            r8 = pers.tile([8, NH], BF16, name="r8")
            nc.vector.tensor_mul(r8[:], rec8[:], mi8_sb[:])

            ita = [bigp.tile([128, NH], F32, name=f"ita{c}", tag="big") for c in range(4)]
            for h in range(HEADS):
                kc, ko = h // 2, (h % 2) * 64
                r0 = rowsp.tile([1, NH], BF16, name="r0", tag="r0")
                nc.gpsimd.dma_start(r0[:], r8[h:h + 1, :])
                sps = psml.tile([64, NH], F32, name="sps", tag="s")
                nc.tensor.matmul(sps[:], lhsT=ones_row[0:1, 0:64], rhs=r0[:],
                                 start=True, stop=True)
                nc.vector.tensor_mul(ita[kc][ko:ko + 64, :], pv_sb[h][0:64, :], sps[:])

            # ---- final: itf = ita + cv, then out = itf^T @ W_out + b_out ----
            ops = []
            for ic in range(4):
                pool = pacc if ic < 2 else psml
                t = pool.tile([128, D], F32, name=f"ops{ic}", tag="a" if ic < 2 else "s")
                nc.tensor.matmul(t[:], lhsT=ones_row[0:1, 0:128], rhs=bout_row,
                                 start=True, stop=False)
                ops.append(t)
            itf = [bigp.tile([128, NH], BF16, name=f"itf{c}", tag="big") for c in range(4)]
            for c2 in range(4):
                nc.vector.tensor_add(itf[c2][:], ita[c2][:], cvs[c2][:])
                for ic in range(4):
                    nc.tensor.matmul(ops[ic][:], lhsT=itf[c2][:, ts(ic, 128)],
                                     rhs=wout_sb[c2][:], start=False, stop=(c2 == 3))
            for ic in range(4):
                osb = outp.tile([128, D], F32, name="osb", tag="osb")
                nc.vector.tensor_copy(osb[:], ops[ic][:])
                nc.sync.dma_start(out[ts(ic, 128), :], osb[:])

    nc.compile()
    return nc


def get_nc():
    if "nc" not in _CACHE:
        _CACHE["nc"] = _build_nc()
    return _CACHE["nc"]


def make_in_maps(x, mask, adjacency_mat, distance_mat, W_qkv, W_out, b_out):
    x = np.ascontiguousarray(np.asarray(x, np.float32))
    mask = np.asarray(mask)
    adjacency_mat = np.asarray(adjacency_mat, np.float32)
    distance_mat = np.asarray(distance_mat, np.float32)
    W_qkv = np.asarray(W_qkv, np.float32)
    W_out_b = np.ascontiguousarray(np.asarray(W_out, np.float32)).astype(BF)
    b_out = np.asarray(b_out, np.float32)

    W3 = W_qkv.reshape(D, HEADS, 3, DH)
    wq = np.ascontiguousarray(W3[:, :, 0, :].reshape(D, D)).astype(BF)
    wk = np.ascontiguousarray(W3[:, :, 1, :].reshape(D, D)).astype(BF)
    wv = np.ascontiguousarray(W3[:, :, 2, :].reshape(D, D)).astype(BF)

    xT = [np.ascontiguousarray(x[b].T).astype(BF) for b in range(B)]
    wv_f32 = W3[:, :, 2, :].reshape(D, D).astype(np.float32)
    u_host = [(LA / N) * (x[b].sum(0) @ wv_f32) for b in range(B)]

    in_maps = []
    for core in range(NCORES):
        b, half = core // 2, core % 2
        i0 = half * NH
        mj = mask[b].astype(np.float32)
        mi = mask[b, i0:i0 + NH].astype(np.float32)

        biasj = np.where(mj > 0, np.float32(np.log(LA)), np.float32(NEG))
        lnldmi = np.where(mi > 0, np.float32(np.log(LD)), np.float32(NEG))
        lgmi = (LG * mi).astype(np.float32)

        cvec = np.zeros((128, 24), np.float32)
        cvec[:, 0:8] = biasj.reshape(8, 128).T
        cvec[:, 8:16] = mj.reshape(8, 128).T
        cvec[:, 16:20] = lnldmi.reshape(4, 128).T
        cvec[:, 20:24] = lgmi.reshape(4, 128).T

        rvec = np.zeros((1, 1664), np.float32)
        rvec[0, 0:NH] = 1.0 - mi
        rvec[0, NH:NH + D] = b_out
        rvec[0, 2 * NH:2 * NH + 128] = 1.0
        rvec[0, 2 * NH + 128:2 * NH + 128 + D] = u_host[b]

        # mi8 also carries the exact correction for the bf16-rounded 1/LA
        # ones-column: computed softmax part = LA*P/((LA*(1/LA)_bf16)*sumP)
        corr = LA * INV_LA_BF
        mi8v = (np.tile(mi[None, :], (8, 1)) * corr).astype(np.float32)

        in_maps.append({
            "xT": xT[b],
            "xTq": np.ascontiguousarray(x[b, i0:i0 + NH, :].T).astype(BF),
            "wq": wq, "wk": wk, "wv": wv,
            "wout": W_out_b,
            "adj": np.ascontiguousarray(adjacency_mat[b, i0:i0 + NH, :]),
            "dist": np.ascontiguousarray(distance_mat[b, i0:i0 + NH, :]),
            "cvec": cvec,
            "rvec": rvec.astype(BF),
            "mi8": mi8v,
        })
    return in_maps


def kernel(x, mask, adjacency_mat, distance_mat, W_qkv, W_out, b_out):
    from concourse.bass_utils import run_bass_kernel_spmd

    nc = get_nc()
    in_maps = make_in_maps(x, mask, adjacency_mat, distance_mat, W_qkv, W_out, b_out)
    res = run_bass_kernel_spmd(nc, in_maps, core_ids=list(range(NCORES)))
    out_full = np.zeros((B, N, D), np.float32)
    for core in range(NCORES):
        b, half = core // 2, core % 2
        out_full[b, half * NH:(half + 1) * NH, :] = res.results[core]["out"]
    return out_full
